# revision 5
# baseline (speedup 1.0000x reference)
"""CGCNN regressor forward pass on 8 Trainium2 NeuronCores (Bass/Tile).

Sharding: data-parallel over destination nodes (6250/core, padded to
6272 = 49*128). Each core owns the edges of its dst nodes, sorted into 49
windows of 128 dst nodes; window edges are split lo/hi by src half so V
gathers index int16 tables <= 25088 rows.

Per layer the message pre-activation z^T [128=2*EMB feats, slots] is built
feature-major in PSUM from three matmuls per 512-slot chunk:
    z^T = CC^T ea        (lhsT=CC shared per layer)
        + U_win^T ohT    (lhsT=U own-window rows in SBUF, rhs=fp8 onehot^T)
        + I V^T           (V^T slabs from bf16 transpose-mode dma_gather)
Layer 0's U+V term is host-precomputed per slot (embedding lookups) and
streamed, so layer 0 runs no gathers. sigma/softplus use the exp+ln table
(pinned via explicit InstLoadActFuncSet to stop table thrashing) plus a DVE
approx-reciprocal. Messages m^T are PE-transposed back per 128-slot tile and
scatter-added via matmul with streamed fp8 onehots into agg^T per window.
BN stats AllReduce [64,2] fp32, agg AllGather bf16, x update (affine +
residual + SiLU) maintains a replicated bf16 X^T slab and the next layer's
U (SBUF) / V (DRAM bf16) tables are rebuilt on device. Mean-pool via
onehot(batch) matmul + AllReduce; 2-layer head replicated; core 0 returns.
"""

import numpy as np
import ml_dtypes

N_NODES = 50000
N_EDGES = 800000
N_GRAPHS = 256
EMB = 64
EDGE_DIM = 50
N_CONVS = 3
HIDDEN = 128
VOCAB = 119
BN_EPS = 1e-5

NCORES = 8
RPC = N_NODES // NCORES          # 6250 real nodes per core
NGRP = 49                        # dst windows per core
NPC = NGRP * 128                 # 6272 padded nodes per core
STRIPE = 4 * NPC                 # 25088 (nodes per partition stripe / lo half)
NPAD = 8 * NPC                   # 50176 padded global nodes
FF = 2 * EMB                     # 128 (f||s)
KEA = EDGE_DIM + 1               # 51 (edge_attr + ones row)

BF = ml_dtypes.bfloat16
F8 = ml_dtypes.float8_e4m3fn

_BUILD_CACHE = {}


def _ceil128(x):
    return (int(x) + 127) // 128 * 128


def _wrap16(idx):
    n = len(idx)
    assert n % 16 == 0
    w = idx.reshape(n // 16, 16).T
    return np.tile(w, (8, 1)).astype(np.int16)


def _pad_to_global(n):
    return NPC * (n // RPC) + (n % RPC)


def _host_prep(z, edge_index, edge_attr, batch, node_emb, AA0, BB0):
    z = np.clip(np.asarray(z), 0, VOCAB - 1).astype(np.int64)
    src = np.asarray(edge_index[0]).astype(np.int64)
    dst = np.asarray(edge_index[1]).astype(np.int64)
    ea = np.asarray(edge_attr, dtype=np.float32)
    batch = np.asarray(batch).astype(np.int64)
    emb = np.asarray(node_emb, np.float32)

    core = dst // RPC
    loc = dst % RPC
    grp = loc // 128
    dloc = loc % 128
    src_pad = _pad_to_global(src)
    src_hi = (src_pad >= STRIPE).astype(np.int64)

    order = np.lexsort((src_hi, grp, core))
    key = (core[order] * NGRP + grp[order]) * 2 + src_hi[order]
    cnt = np.bincount(key, minlength=NCORES * NGRP * 2).reshape(
        NCORES, NGRP, 2)
    Lcnt, Hcnt = cnt[:, :, 0], cnt[:, :, 1]

    Lstar = np.array([_ceil128(Lcnt[:, g].max()) for g in range(NGRP)])
    Tg = np.array([_ceil128(Lstar[g] + Hcnt[:, g].max()) // 128
                   for g in range(NGRP)])
    toff = np.concatenate([[0], np.cumsum(Tg)])[:-1]
    Ttot = int(Tg.sum())
    nslots = Ttot * 128

    ends = np.cumsum(cnt.reshape(-1))
    run_starts = np.concatenate([[0], ends[:-1]]).reshape(NCORES, NGRP, 2)

    slot_edge = np.full((NCORES, nslots), -1, np.int64)
    for c in range(NCORES):
        for g in range(NGRP):
            s0 = int(toff[g]) * 128
            ls, hs = run_starts[c, g]
            lN, hN = int(Lcnt[c, g]), int(Hcnt[c, g])
            slot_edge[c, s0:s0 + lN] = order[ls:ls + lN]
            s1 = s0 + int(Lstar[g])
            slot_edge[c, s1:s1 + hN] = order[hs:hs + hN]

    # layer-0 per-node message tables (host constant-fold of emb @ AA/BB)
    EA0 = emb @ AA0              # [VOCAB, FF] (dst half, already negated-f)
    EB0 = emb @ BB0              # [VOCAB, FF] (src half)
    x0 = emb[z]                  # [N, EMB]

    iota256 = np.broadcast_to(
        np.arange(256, dtype=np.float32), (128, 256)).copy()

    slots = np.arange(nslots)
    srow = (slots % 128).astype(np.int64)      # partition of slot
    scol_t = (slots // 128).astype(np.int64)   # tile of slot

    per_core = []
    for c in range(NCORES):
        se = slot_edge[c]
        valid = se >= 0
        e_ids = np.where(valid, se, 0)

        eaT = np.zeros((KEA, nslots), BF)
        eaT[:EDGE_DIM, :] = np.where(
            valid[None, :], ea[e_ids].T, 0.0).astype(BF)
        eaT[EDGE_DIM, :] = np.float32(1.0)

        dl = dloc[e_ids]
        # oh[p, t*128+d] = (slot t*128+p has dloc d); zero rows for invalid
        oh = np.zeros((128, nslots), F8)
        vs = slots[valid]
        oh[srow[vs], scol_t[vs] * 128 + dl[vs]] = 1.0
        # ohT[d, slot] = (dloc(slot) == d)
        ohT = np.zeros((128, nslots), F8)
        ohT[dl[vs], vs] = 1.0

        # layer-0 U+V slab, transposed: [FF, slot]
        suv = np.zeros((128, nslots), np.float32)
        suv[:, vs] = (EA0[z[dst[e_ids[vs]]]] + EB0[z[src[e_ids[vs]]]]).T
        suv0 = suv.astype(BF)

        lo_parts, hi_parts = [], []
        for g in range(NGRP):
            s0 = int(toff[g]) * 128
            s1 = s0 + int(Lstar[g])
            s2 = int(toff[g] + Tg[g]) * 128
            sl = se[s0:s1]
            lo_parts.append(np.where(
                sl >= 0, src_pad[np.where(sl >= 0, sl, 0)], 0
            ).astype(np.int16))
            sh = se[s1:s2]
            hi_parts.append(np.where(
                sh >= 0, src_pad[np.where(sh >= 0, sh, 0)] - STRIPE, 0
            ).astype(np.int16))
        idx_srclo = _wrap16(np.concatenate(lo_parts))
        idx_srchi = _wrap16(np.concatenate(hi_parts))

        own_real = np.arange(RPC) + c * RPC
        bl = np.full(NPC, 300.0, np.float32)
        bl[:RPC] = batch[own_real]
        bloc = np.ascontiguousarray(bl.reshape(NGRP, 128).T)

        xo = np.zeros((EMB, NPC), np.float32)
        xo[:, :RPC] = x0[own_real].T
        xown0 = xo.astype(BF)

        per_core.append(dict(
            eaT=eaT, oh=oh, ohT=ohT, suv0=suv0,
            isrclo=idx_srclo, isrchi=idx_srchi,
            bloc=bloc, xown0=xown0,
        ))

    # replicated x0^T slab [128, STRIPE]: row 64*s+f, col j = node s*STRIPE+j
    x0pad = np.zeros((NPAD, EMB), np.float32)
    x0pad[_pad_to_global(np.arange(N_NODES))] = x0
    XT0 = np.concatenate(
        [x0pad[:STRIPE].T, x0pad[STRIPE:].T], axis=0).astype(BF)

    shared = dict(iota256=iota256, XT0=np.ascontiguousarray(XT0),
                  id128b=np.eye(128).astype(BF),
                  id128f=np.eye(128).astype(np.float32))
    meta = dict(Tg=tuple(int(t) for t in Tg),
                Lstar=tuple(int(x) for x in Lstar), Ttot=Ttot)
    return meta, per_core, shared


def _prep_weights(node_emb, Wf, bf, Ws, bs, gamma, beta, W1, b1, W2, b2):
    Wf = np.asarray(Wf, np.float32)
    Ws = np.asarray(Ws, np.float32)
    bf = np.asarray(bf, np.float32)
    bs = np.asarray(bs, np.float32)
    CC = np.zeros((N_CONVS, KEA, FF), np.float32)
    AA = np.zeros((N_CONVS, EMB, FF), np.float32)
    BB = np.zeros((N_CONVS, EMB, FF), np.float32)
    for l in range(N_CONVS):
        CC[l, :EDGE_DIM, :EMB] = -Wf[l, 2 * EMB:, :]
        CC[l, :EDGE_DIM, EMB:] = Ws[l, 2 * EMB:, :]
        CC[l, EDGE_DIM, :EMB] = -bf[l]
        CC[l, EDGE_DIM, EMB:] = bs[l]
        AA[l, :, :EMB] = -Wf[l, :EMB, :]
        AA[l, :, EMB:] = Ws[l, :EMB, :]
        BB[l, :, :EMB] = -Wf[l, EMB:2 * EMB, :]
        BB[l, :, EMB:] = Ws[l, EMB:2 * EMB, :]
    BB2 = np.tile(BB, (1, 2, 1))             # [L, 128, FF] stacked stripes
    gamma2 = np.tile(np.asarray(gamma, np.float32), (1, 2))[:, :, None].copy()
    beta2 = np.tile(np.asarray(beta, np.float32), (1, 2))[:, :, None].copy()
    W1b = np.concatenate(
        [np.asarray(W1, np.float32), np.asarray(b1, np.float32)[None, :]], 0)
    b2b = np.full((128, 1), float(np.asarray(b2).reshape(-1)[0]), np.float32)
    return dict(CC=CC.astype(BF), AA=AA.astype(BF), BB2=BB2.astype(BF),
                gamma2=gamma2, beta2=beta2, W1b=W1b,
                W2=np.asarray(W2, np.float32), b2b=b2b), AA[0], BB[0]


def _build(meta):
    import time
    _t0 = time.time()
    import concourse.tile as tile
    from concourse import bacc, mybir
    from concourse.hw_specs import get_activation_tables
    from concourse.dve_ops import (RECIPROCAL_APPROX_FAST,
                                   RECIP_APPROX_FAST_CONSTS)

    AF = mybir.ActivationFunctionType
    ALU = mybir.AluOpType
    F32 = mybir.dt.float32
    BF16 = mybir.dt.bfloat16
    FP8 = mybir.dt.float8e4
    I16 = mybir.dt.int16
    RC = RECIP_APPROX_FAST_CONSTS
    AX = mybir.AxisListType

    Tg = meta["Tg"]
    Lstar = meta["Lstar"]
    Ttot = meta["Ttot"]
    toff = np.concatenate([[0], np.cumsum(Tg)])[:-1].astype(int)
    lo_off = np.concatenate([[0], np.cumsum(Lstar)])[:-1].astype(int)
    LOtot = int(np.sum(Lstar))
    hi_spans = [Tg[g] * 128 - Lstar[g] for g in range(NGRP)]
    hi_off = np.concatenate([[0], np.cumsum(hi_spans)])[:-1].astype(int)
    HItot = int(np.sum(hi_spans))
    TMAX = max(Tg)
    NS = Ttot * 128

    nc = bacc.Bacc("TRN2", target_bir_lowering=False, debug=False,
                   num_devices=NCORES)

    # table id for the exp+ln table set
    tables = list(get_activation_tables("gen3").keys())
    EXPLN_SET = tables.index("natural_log_exp_and_others")

    def pin_act_table(set_id=EXPLN_SET):
        nc.scalar.add_instruction(mybir.InstLoadActFuncSet(
            name=nc.scalar.bass.get_next_instruction_name(),
            ins=[], outs=[], act_func_set_id=set_id))

    def din(name, shape, dt=F32):
        return nc.dram_tensor(name, shape, dt, kind="ExternalInput").ap()

    d_eaT = din("eaT", [KEA, NS], BF16)
    d_oh = din("oh", [128, NS], FP8)
    d_ohT = din("ohT", [128, NS], FP8)
    d_suv0 = din("suv0", [128, NS], BF16)
    d_isrclo = din("isrclo", [128, LOtot // 16], I16)
    d_isrchi = din("isrchi", [128, max(HItot // 16, 1)], I16)
    d_bloc = din("bloc", [128, NGRP])
    d_xown0 = din("xown0", [EMB, NPC], BF16)
    d_XT0 = din("XT0", [128, STRIPE], BF16)
    d_iota256 = din("iota256", [128, 256])
    d_id128b = din("id128b", [128, 128], BF16)
    d_id128f = din("id128f", [128, 128])
    d_CC = din("CC", [N_CONVS, KEA, FF], BF16)
    d_AA = din("AA", [N_CONVS, EMB, FF], BF16)
    d_BB2 = din("BB2", [N_CONVS, 128, FF], BF16)
    d_gamma2 = din("gamma2", [N_CONVS, 128, 1])
    d_beta2 = din("beta2", [N_CONVS, 128, 1])
    d_W1b = din("W1b", [EMB + 1, HIDDEN])
    d_W2 = din("W2", [HIDDEN, 1])
    d_b2b = din("b2b", [128, 1])

    d_yhat = nc.dram_tensor("yhat", [256, 1], F32, kind="ExternalOutput").ap()

    d_V = nc.dram_tensor("V", [NPAD, FF], BF16, kind="Internal").ap()
    d_statin = nc.dram_tensor("statin", [EMB, 2], F32, kind="Internal").ap()
    d_statout = nc.dram_tensor("statout", [EMB, 2], F32, kind="Internal",
                               addr_space="Shared").ap()
    d_aggin = nc.dram_tensor("aggin", [EMB, NPC], BF16, kind="Internal").ap()
    d_aggout = nc.dram_tensor("aggout", [NCORES, EMB, NPC], BF16,
                              kind="Internal", addr_space="Shared").ap()
    d_poolin = nc.dram_tensor("poolin", [EMB + 1, 256], F32,
                              kind="Internal").ap()
    d_poolout = nc.dram_tensor("poolout", [EMB + 1, 256], F32,
                               kind="Internal", addr_space="Shared").ap()

    GROUPS = [list(range(NCORES))]

    XT = nc.alloc_sbuf_tensor("XT", [128, STRIPE], BF16).ap()
    xown = nc.alloc_sbuf_tensor("xown", [EMB, NPC], BF16).ap()
    uown = nc.alloc_sbuf_tensor("uown", [128, NGRP, FF], BF16).ap()

    with tile.TileContext(nc) as tc:
        with (
            tc.tile_pool(name="const", bufs=1) as cpool,
            tc.tile_pool(name="work", bufs=2) as pool,
            tc.tile_pool(name="chunk", bufs=3) as chk,
            tc.tile_pool(name="psum", bufs=2, space="PSUM") as psum,
            tc.tile_pool(name="psmall", bufs=2, space="PSUM") as psm,
        ):
            # ---------------- constants ----------------
            idb_t = cpool.tile([128, 128], BF16)
            nc.sync.dma_start(idb_t[:], d_id128b[:])
            idf_t = cpool.tile([128, 128], F32)
            nc.sync.dma_start(idf_t[:], d_id128f[:])
            iota256_t = cpool.tile([128, 256], F32)
            nc.sync.dma_start(iota256_t[:], d_iota256[:])
            bloc_t = cpool.tile([128, NGRP], F32)
            nc.sync.dma_start(bloc_t[:], d_bloc[:])
            cc_t = []
            aa_t = []
            bb2_t = []
            g2_t = []
            be_t = []
            for l in range(N_CONVS):
                c1 = cpool.tile([KEA, FF], BF16, name=f"cc{l}")
                nc.sync.dma_start(c1[:], d_CC[l])
                cc_t.append(c1)
                a1 = cpool.tile([EMB, FF], BF16, name=f"aa{l}")
                nc.sync.dma_start(a1[:], d_AA[l])
                aa_t.append(a1)
                b1_ = cpool.tile([128, FF], BF16, name=f"bb2{l}")
                nc.sync.dma_start(b1_[:], d_BB2[l])
                bb2_t.append(b1_)
                g1 = cpool.tile([128, 1], F32, name=f"g2{l}")
                nc.sync.dma_start(g1[:], d_gamma2[l])
                g2_t.append(g1)
                bt1 = cpool.tile([128, 1], F32, name=f"be{l}")
                nc.sync.dma_start(bt1[:], d_beta2[l])
                be_t.append(bt1)
            ones1_t = cpool.tile([128, 1], F32)
            nc.gpsimd.memset(ones1_t[:], 1.0)
            w1b_t = cpool.tile([EMB + 1, HIDDEN], F32)
            nc.sync.dma_start(w1b_t[:], d_W1b[:])
            w2_t = cpool.tile([HIDDEN, 1], F32)
            nc.sync.dma_start(w2_t[:], d_W2[:])
            b2b_t = cpool.tile([128, 1], F32)
            nc.sync.dma_start(b2b_t[:], d_b2b[:])

            # ---------------- x0 prelude ----------------
            nc.sync.dma_start(XT[:], d_XT0[:])
            nc.sync.dma_start(xown[:], d_xown0[:])

            # ---------------- conv layers ----------------
            for l in range(N_CONVS):
                pin_act_table()
                s_acc = pool.tile([EMB, 2], F32, tag="sacc", bufs=1)
                nc.gpsimd.memset(s_acc[:], 0.0)

                for g in range(NGRP):
                    T = Tg[g]
                    W = T * 128
                    bt = int(toff[g])
                    nlo = int(Lstar[g])
                    nhi = W - nlo

                    eat = pool.tile([KEA, TMAX * 128], BF16, tag="eat")
                    nc.sync.dma_start(eat[:, :W],
                                      d_eaT[:, bt * 128:bt * 128 + W])
                    ohs = pool.tile([128, TMAX * 128], FP8, tag="ohs")
                    nc.sync.dma_start(ohs[:, :W],
                                      d_oh[:, bt * 128:bt * 128 + W])
                    if l == 0:
                        suv = pool.tile([128, 1, TMAX * 128], BF16, tag="slV")
                        nc.sync.dma_start(suv[:, 0, :W],
                                          d_suv0[:, bt * 128:bt * 128 + W])
                    else:
                        oht = pool.tile([128, TMAX * 128], FP8, tag="oht")
                        nc.sync.dma_start(oht[:, :W],
                                          d_ohT[:, bt * 128:bt * 128 + W])
                        suv = pool.tile([128, 1, TMAX * 128], BF16, tag="slV")
                        ilo = pool.tile([128, TMAX * 8], I16, tag="iq1")
                        lo0 = int(lo_off[g])
                        nc.sync.dma_start(
                            ilo[:, :nlo // 16],
                            d_isrclo[:, lo0 // 16:(lo0 + nlo) // 16])
                        nc.gpsimd.dma_gather(
                            suv[:, :, 0:nlo], d_V[0:STRIPE, :],
                            ilo[:, :nlo // 16], nlo, nlo, FF,
                            elem_step=FF, transpose=True,
                            single_packet=(nlo <= 1024))
                        ihi = pool.tile([128, TMAX * 8], I16, tag="iq2")
                        hi0 = int(hi_off[g])
                        nc.sync.dma_start(
                            ihi[:, :nhi // 16],
                            d_isrchi[:, hi0 // 16:(hi0 + nhi) // 16])
                        nc.gpsimd.dma_gather(
                            suv[:, :, nlo:W], d_V[STRIPE:NPAD, :],
                            ihi[:, :nhi // 16], nhi, nhi, FF,
                            elem_step=FF, transpose=True,
                            single_packet=(nhi <= 1024))

                    pagg = psm.tile([EMB, 128], F32, tag="pagg")
                    ntile = 0
                    for c0 in range(0, W, 512):
                        w = min(512, W - c0)
                        preT = psum.tile([128, 512], F32, tag="pre")
                        nc.tensor.matmul(
                            out=preT[:, :w], lhsT=cc_t[l][:],
                            rhs=eat[:, c0:c0 + w], start=True, stop=False)
                        if l > 0:
                            nc.tensor.matmul(
                                out=preT[:, :w], lhsT=uown[:, g, :],
                                rhs=oht[:, c0:c0 + w], start=False,
                                stop=False)
                        nc.tensor.matmul(
                            out=preT[:, :w], lhsT=idb_t[:],
                            rhs=suv[:, 0, c0:c0 + w], start=False, stop=True)

                        eT = chk.tile([128, 512], BF16, tag="eT")
                        nc.scalar.activation(eT[:, :w], preT[:, :w], AF.Exp)
                        spt = chk.tile([EMB, 512], BF16, tag="spt")
                        nc.scalar.activation(spt[:, :w], eT[EMB:128, :w],
                                             AF.Ln, bias=1.0)
                        den = chk.tile([EMB, 512], F32, tag="den")
                        nc.vector.tensor_scalar_add(
                            den[:, :w], eT[0:EMB, :w], 1.0)
                        nc.vector._custom_dve(
                            RECIPROCAL_APPROX_FAST, out=den[:, :w],
                            in0=den[:, :w], s0=RC["s0"], s1=RC["s1"],
                            imm2=RC["imm2"])
                        mt = chk.tile([EMB, 512], BF16, tag="mt")
                        nc.vector.tensor_mul(mt[:, :w], den[:, :w],
                                             spt[:, :w])
                        for t in range(w // 128):
                            tp = psm.tile([128, EMB], BF16, tag="tp")
                            nc.tensor.transpose(
                                out=tp[:], in_=mt[:, t * 128:(t + 1) * 128],
                                identity=idb_t[0:EMB, 0:EMB])
                            msb = chk.tile([128, EMB], BF16, tag="msb",
                                           bufs=4)
                            nc.vector.tensor_copy(msb[:], tp[:])
                            nc.tensor.matmul(
                                out=pagg[:], lhsT=msb[:],
                                rhs=ohs[:, c0 + t * 128:c0 + (t + 1) * 128],
                                start=(ntile == 0), stop=(ntile == T - 1),
                                skip_group_check=True)
                            ntile += 1

                    # stats + aggin write straight from PSUM
                    red = pool.tile([EMB, 2], F32, tag="red")
                    nc.vector.tensor_reduce(red[:, 0:1], pagg[:], axis=AX.X,
                                            op=ALU.add)
                    sq = pool.tile([EMB, 128], F32, tag="sq")
                    nc.scalar.activation(sq[:], pagg[:], AF.Square,
                                         accum_out=red[:, 1:2])
                    nc.vector.tensor_add(s_acc[:], s_acc[:], red[:])
                    a16 = pool.tile([EMB, 128], BF16, tag="a16")
                    nc.vector.tensor_copy(a16[:], pagg[:])
                    nc.sync.dma_start(
                        d_aggin[:, g * 128:(g + 1) * 128], a16[:])

                # ---- stats AllReduce; bn scalars ----
                nc.sync.dma_start(d_statin[:], s_acc[:])
                nc.gpsimd.collective_compute(
                    "AllReduce", ALU.add, replica_groups=GROUPS,
                    ins=[d_statin[:]], outs=[d_statout[:]])
                st = pool.tile([128, 2], F32, tag="st", bufs=1)
                nc.sync.dma_start(st[0:EMB, :], d_statout[:])
                nc.sync.dma_start(st[EMB:128, :], d_statout[:])
                mu = pool.tile([128, 1], F32, tag="mu", bufs=1)
                nc.vector.tensor_scalar_mul(mu[:], st[:, 0:1], 1.0 / N_NODES)
                var = pool.tile([128, 1], F32, tag="var", bufs=1)
                nc.vector.tensor_scalar_mul(var[:], st[:, 1:2], 1.0 / N_NODES)
                msq = pool.tile([128, 1], F32, tag="msq", bufs=1)
                nc.vector.tensor_mul(msq[:], mu[:], mu[:])
                nc.vector.tensor_tensor(out=var[:], in0=var[:], in1=msq[:],
                                        op=ALU.subtract)
                rsq = pool.tile([128, 1], F32, tag="rsq", bufs=1)
                nc.vector.tensor_scalar_add(var[:], var[:], BN_EPS)
                nc.scalar.activation(rsq[:], var[:], AF.Ln)
                nc.scalar.activation(rsq[:], rsq[:], AF.Exp, scale=-0.5)
                gb1 = pool.tile([128, 1], F32, tag="gb1", bufs=1)
                nc.vector.tensor_mul(gb1[:], g2_t[l][:], rsq[:])
                gb0 = pool.tile([128, 1], F32, tag="gb0", bufs=1)
                nc.vector.tensor_mul(gb0[:], mu[:], gb1[:])
                nc.vector.tensor_tensor(out=gb0[:], in0=be_t[l][:],
                                        in1=gb0[:], op=ALU.subtract)

                # ---- agg AllGather ----
                nc.gpsimd.collective_compute(
                    "AllGather", ALU.bypass, replica_groups=GROUPS,
                    ins=[d_aggin[:]], outs=[d_aggout[:]])

                # ---- replicated x update (into XT) ----
                for b in range(4):
                    for off in range(0, NPC, 512):
                        w = min(512, NPC - off)
                        ag = pool.tile([128, 512], BF16, tag="ag")
                        nc.sync.dma_start(ag[0:EMB, :w],
                                          d_aggout[b, :, off:off + w])
                        nc.sync.dma_start(ag[EMB:128, :w],
                                          d_aggout[b + 4, :, off:off + w])
                        nc.vector.tensor_scalar(
                            out=ag[:, :w], in0=ag[:, :w], scalar1=gb1[:],
                            scalar2=gb0[:], op0=ALU.mult, op1=ALU.add)
                        cols = b * NPC + off
                        nc.vector.tensor_add(ag[:, :w], ag[:, :w],
                                             XT[:, cols:cols + w])
                        nc.scalar.activation(XT[:, cols:cols + w],
                                             ag[:, :w], AF.Silu)

                # ---- own x update ----
                for off in range(0, NPC, 512):
                    w = min(512, NPC - off)
                    ag = pool.tile([EMB, 512], BF16, tag="ago")
                    nc.sync.dma_start(ag[:, :w], d_aggin[:, off:off + w])
                    nc.vector.tensor_scalar(
                        out=ag[:, :w], in0=ag[:, :w], scalar1=gb1[0:EMB],
                        scalar2=gb0[0:EMB], op0=ALU.mult, op1=ALU.add)
                    nc.vector.tensor_add(ag[:, :w], ag[:, :w],
                                         xown[:, off:off + w])
                    nc.scalar.activation(xown[:, off:off + w], ag[:, :w],
                                         AF.Silu)

                # ---- next-layer U (SBUF) and V (DRAM) tables ----
                if l + 1 < N_CONVS:
                    for nt4 in range(NPAD // 512):
                        pt = psum.tile([128, 512], F32, tag="pre")
                        for j in range(4):
                            nt = nt4 * 4 + j
                            s = nt // (STRIPE // 128)
                            col = (nt % (STRIPE // 128)) * 128
                            nc.tensor.matmul(
                                out=pt[:, j * FF:(j + 1) * FF],
                                lhsT=XT[s * EMB:(s + 1) * EMB,
                                        col:col + 128],
                                rhs=bb2_t[l + 1][s * EMB:(s + 1) * EMB, :],
                                start=True, stop=True)
                        vb = pool.tile([128, 512], BF16, tag="vb")
                        nc.vector.tensor_copy(vb[:], pt[:])
                        nc.sync.dma_start(
                            d_V[nt4 * 512:(nt4 + 1) * 512, :].rearrange(
                                "(j p) f -> p j f", p=128), vb[:])
                    for g in range(NGRP):
                        pu = psum.tile([128, 512], F32, tag="pre")
                        nc.tensor.matmul(
                            out=pu[:, 0:FF],
                            lhsT=xown[:, g * 128:(g + 1) * 128],
                            rhs=aa_t[l + 1][:], start=True, stop=True)
                        nc.vector.tensor_copy(uown[:, g, :], pu[:, 0:FF])

            # ---------------- pooling ----------------
            pp = psum.tile([128, 256], F32, tag="pre")
            for t in range(NGRP):
                tp = psm.tile([128, EMB], BF16, tag="tp")
                nc.tensor.transpose(out=tp[:],
                                    in_=xown[:, t * 128:(t + 1) * 128],
                                    identity=idb_t[0:EMB, 0:EMB])
                xr = pool.tile([128, EMB + 1], F32, tag="xr")
                nc.vector.tensor_copy(xr[:, 0:EMB], tp[:])
                nc.gpsimd.memset(xr[:, EMB:EMB + 1], 1.0)
                oh2 = pool.tile([128, 256], F32, tag="oh2")
                nc.vector.tensor_scalar(
                    out=oh2[:], in0=iota256_t[:],
                    scalar1=bloc_t[:, t:t + 1], scalar2=None,
                    op0=ALU.is_equal)
                nc.tensor.matmul(out=pp[0:EMB + 1, :], lhsT=xr[:],
                                 rhs=oh2[:], start=(t == 0),
                                 stop=(t == NGRP - 1),
                                 skip_group_check=True)
            psb = pool.tile([128, 256], F32, tag="psb")
            nc.vector.tensor_copy(psb[0:EMB + 1, :], pp[0:EMB + 1, :])
            nc.sync.dma_start(d_poolin[:], psb[0:EMB + 1, :])
            nc.gpsimd.collective_compute(
                "AllReduce", ALU.add, replica_groups=GROUPS,
                ins=[d_poolin[:]], outs=[d_poolout[:]])
            pT = pool.tile([EMB + 1, 256], F32, tag="pT")
            nc.sync.dma_start(pT[:], d_poolout[:])

            # ---------------- head (replicated) ----------------
            for h in range(2):
                sl = slice(h * 128, (h + 1) * 128)
                ctp = psm.tile([128, 128], F32, tag="ctp")
                nc.tensor.transpose(out=ctp[:, 0:1],
                                    in_=pT[EMB:EMB + 1, sl],
                                    identity=ones1_t[EMB:EMB + 1, 0:1])
                cnt = pool.tile([128, 1], F32, tag="cnt")
                nc.vector.tensor_copy(cnt[:], ctp[:, 0:1])
                nc.vector.tensor_scalar_max(cnt[:], cnt[:], 1.0)
                nc.vector._custom_dve(
                    RECIPROCAL_APPROX_FAST, out=cnt[:], in0=cnt[:],
                    s0=RC["s0"], s1=RC["s1"], imm2=RC["imm2"])
                hp = psum.tile([128, 512], F32, tag="pre")
                nc.tensor.matmul(out=hp[:, 0:HIDDEN], lhsT=pT[:, sl],
                                 rhs=w1b_t[:], start=True, stop=True)
                hs = pool.tile([128, HIDDEN], F32, tag="hs")
                nc.vector.tensor_scalar_mul(hs[:], hp[:, 0:HIDDEN], cnt[:])
                nc.scalar.activation(hs[:], hs[:], AF.Silu)
                htp = psm.tile([128, 128], F32, tag="ctp")
                nc.tensor.transpose(out=htp[:], in_=hs[:],
                                    identity=idf_t[:])
                hT = pool.tile([128, 128], F32, tag="hT")
                nc.vector.tensor_copy(hT[:], htp[:])
                yp = psm.tile([128, 128], F32, tag="ctp")
                nc.tensor.matmul(out=yp[:, 0:1], lhsT=hT[:],
                                 rhs=w2_t[:], start=True, stop=True)
                yv = pool.tile([128, 1], F32, tag="yv")
                nc.vector.tensor_add(yv[:], yp[0:128, 0:1], b2b_t[:])
                nc.sync.dma_start(d_yhat[sl, :], yv[:])

    _t1 = time.time()
    nc.compile()
    print(f"[kernel] trace {_t1 - _t0:.0f}s  bacc-compile "
          f"{time.time() - _t1:.0f}s", flush=True)
    return nc


def _install_ntff_hook():
    """Dev-only: register the NTFF profiling hook (antenv.axon_hooks shim)
    so run_bass_kernel_spmd(trace=True) works under axon in this image."""
    import sys
    import types
    import ctypes
    import contextlib
    if "antenv.axon_hooks" in sys.modules:
        return
    so_path = "/opt/axon/libaxon_pjrt.so"
    lib = ctypes.CDLL(so_path)
    if not hasattr(lib, "axon_start_nrt_profile"):
        return
    lib.axon_start_nrt_profile.argtypes = [
        ctypes.POINTER(ctypes.c_int64), ctypes.c_size_t]
    lib.axon_start_nrt_profile.restype = ctypes.c_int64
    lib.axon_stop_nrt_profile.argtypes = [ctypes.c_char_p]
    lib.axon_stop_nrt_profile.restype = ctypes.c_int64

    @contextlib.contextmanager
    def _hook(output_dir, device_ids):
        import jax
        jax.devices()
        if device_ids:
            ids = (ctypes.c_int64 * len(device_ids))(*device_ids)
            rc = lib.axon_start_nrt_profile(ids, len(device_ids))
        else:
            rc = lib.axon_start_nrt_profile(None, 0)
        if rc != 0:
            raise RuntimeError(f"axon_start_nrt_profile rc={rc}")
        try:
            yield
        finally:
            n = lib.axon_stop_nrt_profile(str(output_dir).encode())
            print(f"[ktrace] {n} profile file(s) -> {output_dir}", flush=True)

    mod = types.ModuleType("antenv.axon_hooks")
    mod.get_axon_ntff_profile_hook = lambda: _hook
    mod.set_axon_ntff_profile_hook = lambda h: None
    sys.modules["antenv.axon_hooks"] = mod


LAST_EXEC_NS = None
LAST_TRACE = None
LAST_INSTS = None


def kernel(z, edge_index, edge_attr, batch, node_emb, Wf, bf, Ws, bs,
           gamma, beta, W1, b1, W2, b2):
    import os
    from concourse import bass_utils

    wts, AA0, BB0 = _prep_weights(node_emb, Wf, bf, Ws, bs, gamma, beta,
                                  W1, b1, W2, b2)
    meta, per_core, shared = _host_prep(z, edge_index, edge_attr, batch,
                                        node_emb, AA0, BB0)

    key = (meta["Tg"], meta["Lstar"])
    if key not in _BUILD_CACHE:
        _BUILD_CACHE[key] = _build(meta)
    nc = _BUILD_CACHE[key]

    in_maps = []
    for c in range(NCORES):
        m = dict(per_core[c])
        m.update(shared)
        m.update(wts)
        in_maps.append(m)

    kw = {}
    if os.environ.get("KTRACE"):
        _install_ntff_hook()
        kw = dict(trace=True,
                  trace_cores=[int(x) for x in
                               os.environ.get("KTRACE_CORES", "0").split(",")])

    import time as _tm
    _t = _tm.time()
    res = bass_utils.run_bass_kernel_spmd(nc, in_maps,
                                          core_ids=list(range(NCORES)), **kw)
    global LAST_RUN_S, LAST_EXEC_NS, LAST_TRACE, LAST_INSTS
    LAST_RUN_S = _tm.time() - _t
    if res.exec_time_ns:
        LAST_EXEC_NS = res.exec_time_ns
    if res.instructions_and_trace is not None:
        LAST_INSTS = res.instructions_and_trace[0]
        LAST_TRACE = res.instructions_and_trace[1]
    print(f"[kernel] spmd run (compile+exec) {LAST_RUN_S:.1f}s", flush=True)
    return np.asarray(res.results[0]["yhat"]).reshape(256).copy()
